# revision 1
# baseline (speedup 1.0000x reference)
"""Multi-head attention (B=4, S=2048, D=1024, H=16, dh=64) on 8 TRN2 NeuronCores.

Sharding: core c = (batch b, head-group g) with b = c // 2, g = c % 2.
Each core computes heads g*8..g*8+7 for batch b:
    Q/K/V projections (weight columns sliced), per-head softmax attention,
    and a partial o-projection (Wo rows sliced). Host sums the two partials
    per batch and adds bo.

Device-side layout (all matmuls in float32r = TF32-like, full PE rate):
  - x is fed pre-transposed as xT [1024, 2048]; Q^T/K^T computed natively as
    [feat, tok] (lhsT = W tiles), V natively as [tok, feat] (rhs = Wv).
  - scores computed transposed: S^T[k, q] = K_h x Q_h^T contraction over dh=64,
    softmax denominator via an appended ones-column in the PV matmul
    (lhsT = [V_h | 1], M=65) -> no max-subtraction needed (|scores| ~ 4).
  - exp on ScalarE (scale=1/8 folded in), one inst per [128, 1024] (2 PSUM banks).
  - normalize ctx^T by 1/Z via reciprocal + DRAM-bounce partition-broadcast.
  - o-proj: y^T[o, q] = Wo^T ctxn^T accumulated over 4 feature tiles.
"""
import json
import os
import sys

sys.path.insert(0, "/opt/trn_rl_repo")

import numpy as np

import concourse.bass as bass
import concourse.tile as tile
from concourse import mybir

F32 = mybir.dt.float32
F32R = mybir.dt.float32r
EXP = mybir.ActivationFunctionType.Exp

D = 1024          # d_model
S = 2048          # sequence
B = 4             # batch
FT = 512          # features per core (8 heads x 64)
DH = 64
NKT_IN = D // 128     # 8  k-tiles over d_model
NQB = S // 512        # 4  query blocks
NKT = S // 128        # 16 key tiles
NTT = S // 128        # 16 token tiles
SCALE = 1.0 / 8.0     # 1/sqrt(64)
N_CORES = 8


# --------------------------------------------------------------------------
# walrus in this container rejects instructions with >1 sync wait; split the
# extras onto preceding same-engine NoOps (semantically identical).
def _fix_bir_json(bir_bytes):
    j = json.loads(bir_bytes)
    n = 0
    for fn in j["functions"]:
        for blk in fn["blocks"]:
            out = []
            changed = False
            for inst in blk["instructions"]:
                si = inst.get("sync_info")
                waits = si.get("on_wait") if si else None
                if waits and len(waits) > 1:
                    for w in waits[:-1]:
                        n += 1
                        nop = {
                            "name": f"I-wsplit-{n}",
                            "opcode": "NoOp",
                            "engine": inst["engine"],
                            "ins": [],
                            "outs": [],
                            "sync_info": {"on_wait": [w], "on_update": []},
                        }
                        if "debug" in inst:
                            nop["debug"] = inst["debug"]
                        out.append(nop)
                    si["on_wait"] = [waits[-1]]
                    changed = True
                out.append(inst)
            if changed:
                blk["instructions"] = out
    return json.dumps(j).encode()


def _install_compile_patch():
    import concourse.bass_utils as _bu
    import concourse.bass2jax as _b2j

    if getattr(_bu, "_waitfix_installed", False):
        return
    _orig = _bu.compile_bir_kernel

    def _patched(bir_json, tmpdir, neff_name="file.neff"):
        return _orig(_fix_bir_json(bir_json), tmpdir, neff_name)

    _bu.compile_bir_kernel = _patched
    _b2j.compile_bir_kernel = _patched
    _bu._waitfix_installed = True


# --------------------------------------------------------------------------
def _build():
    nc = bass.Bass("TRN2", target_bir_lowering=False, debug=False,
                   enable_asserts=False, num_devices=N_CORES)

    xT = nc.dram_tensor("xT", [D, S], F32, kind="ExternalInput")
    wq = nc.dram_tensor("wq", [D, FT], F32, kind="ExternalInput")
    wk = nc.dram_tensor("wk", [D, FT], F32, kind="ExternalInput")
    wv = nc.dram_tensor("wv", [D, FT], F32, kind="ExternalInput")
    wo = nc.dram_tensor("wo", [FT, D], F32, kind="ExternalInput")
    bq = nc.dram_tensor("bq", [FT], F32, kind="ExternalInput")
    bk = nc.dram_tensor("bk", [FT], F32, kind="ExternalInput")
    # bv interleaved with the ones column: [8 * 65] = per head 64 bias + 1.0
    bvones = nc.dram_tensor("bvones", [8 * 65], F32, kind="ExternalInput")
    yT = nc.dram_tensor("yT", [D, S], F32, kind="ExternalOutput")
    rscr = nc.dram_tensor("rscr", [32, 512], F32, kind="Internal")

    with tile.TileContext(nc) as tc:
        with tc.tile_pool(name="qk_sb", bufs=1) as qk_sb, \
             tc.tile_pool(name="v1_sb", bufs=1) as v1_sb, \
             tc.tile_pool(name="ctxn_sb", bufs=1) as ctxn_sb, \
             tc.tile_pool(name="x_sb", bufs=1) as x_sb, \
             tc.tile_pool(name="w_sb", bufs=1) as w_sb, \
             tc.tile_pool(name="wv_sb", bufs=1) as wv_sb, \
             tc.tile_pool(name="b_sb", bufs=2) as b_sb, \
             tc.tile_pool(name="e_sb", bufs=4) as e_sb, \
             tc.tile_pool(name="r_sb", bufs=2) as r_sb, \
             tc.tile_pool(name="y_sb", bufs=3) as y_sb, \
             tc.tile_pool(name="ps_s", bufs=2, space="PSUM") as ps_s, \
             tc.tile_pool(name="ps_c", bufs=2, space="PSUM") as ps_c:

            # ctx^T accumulator for all 8 heads: 4 tiles [128, 2048] (f32r)
            ctxn = [ctxn_sb.tile([128, S], F32R, tag=f"ctxn{i}", name=f"ctxn{i}") for i in range(4)]

            v1_tiles = None   # current half's V1 tiles (4 heads interleaved | ones)

            for p in range(4):           # pass p: heads 2p, 2p+1 (local)
                # ---- load xT (all 8 k-tiles) and this pass's weights ----
                xts = []
                for kt in range(NKT_IN):
                    t = x_sb.tile([128, S], F32R, tag=f"x{kt}")
                    nc.sync.dma_start(t[:], xT.ap()[kt * 128:(kt + 1) * 128, :].bitcast(F32R))
                    xts.append(t)
                wq_t, wk_t = [], []
                for kt in range(NKT_IN):
                    tq = w_sb.tile([128, 128], F32R, tag=f"wq{kt}")
                    nc.sync.dma_start(tq[:], wq.ap()[kt * 128:(kt + 1) * 128,
                                                     p * 128:(p + 1) * 128].bitcast(F32R))
                    wq_t.append(tq)
                    tk = w_sb.tile([128, 128], F32R, tag=f"wk{kt}")
                    nc.sync.dma_start(tk[:], wk.ap()[kt * 128:(kt + 1) * 128,
                                                     p * 128:(p + 1) * 128].bitcast(F32R))
                    wk_t.append(tk)
                bq_t = b_sb.tile([128, 1], F32, tag="bq")
                nc.sync.dma_start(bq_t[:], bq.ap()[p * 128:(p + 1) * 128][:, None])
                bk_t = b_sb.tile([128, 1], F32, tag="bk")
                nc.sync.dma_start(bk_t[:], bk.ap()[p * 128:(p + 1) * 128][:, None])

                # ---- Q^T, K^T for this pass's 128 features ----
                qt_t = qk_sb.tile([128, S], F32R, tag="qt")
                kt_t = qk_sb.tile([128, S], F32R, tag="kt")
                for (wt, bt, dst) in ((wq_t, bq_t, qt_t), (wk_t, bk_t, kt_t)):
                    for qb in range(NQB):
                        pp = ps_s.tile([128, 512], F32, tag="sp")
                        for kt in range(NKT_IN):
                            nc.tensor.matmul(pp[:], wt[kt][:],
                                             xts[kt][:, qb * 512:(qb + 1) * 512],
                                             start=(kt == 0), stop=(kt == NKT_IN - 1))
                        nc.vector.tensor_scalar_add(
                            dst[:, qb * 512:(qb + 1) * 512], pp[:], bt[:])

                # ---- V for heads 4h..4h+3 on even passes (half at a time) ----
                if p % 2 == 0:
                    half = p // 2
                    wv_t = []
                    for kt in range(NKT_IN):
                        t = wv_sb.tile([128, 256], F32R, tag=f"wv{kt}")
                        nc.sync.dma_start(t[:], wv.ap()[kt * 128:(kt + 1) * 128,
                                                        half * 256:(half + 1) * 256].bitcast(F32R))
                        wv_t.append(t)
                    bb = b_sb.tile([128, 260], F32, tag="bb")
                    nc.gpsimd.dma_start(bb[:], bass.AP(
                        tensor=bvones, offset=half * 260, ap=[[0, 128], [1, 260]]))
                    v1_tiles = [v1_sb.tile([128, 260], F32R, tag=f"v1_{tt}", name=f"v1_{tt}")
                                for tt in range(NTT)]
                    for tt in range(NTT):
                        pv = ps_s.tile([128, 256], F32, tag="sp")
                        for kt in range(NKT_IN):
                            nc.tensor.matmul(pv[:], xts[kt][:, tt * 128:(tt + 1) * 128],
                                             wv_t[kt][:],
                                             start=(kt == 0), stop=(kt == NKT_IN - 1))
                        v1v = v1_tiles[tt][:].rearrange("p (s c) -> p s c", c=65)
                        nc.vector.tensor_add(
                            v1v[:, :, 0:64],
                            pv[:].rearrange("p (s c) -> p s c", c=64),
                            bb[:].rearrange("p (s c) -> p s c", c=65)[:, :, 0:64])
                        nc.sync.dma_start(
                            v1v[:, :, 64:65],
                            bass.AP(tensor=bvones, offset=half * 260 + 64,
                                    ap=[[0, 128], [65, 4], [0, 1]]).bitcast(F32R))

                # ---- attention: both heads row-paired, one exp per kt ----
                segA = (2 * p) % 4
                segB = (2 * p + 1) % 4
                for qb in range(NQB):
                    ctxA = ps_c.tile([65, 512], F32, tag="ctxA")
                    ctxB = ps_c.tile([65, 512], F32, tag="ctxB")
                    for kt in range(NKT):
                        # head A on PE rows 0-63, head B on rows 64-127: the two
                        # scores matmuls use disjoint row groups and stream
                        # concurrently; outputs land in adjacent PSUM banks.
                        sp = ps_s.tile([128, 1024], F32, tag="sp")
                        nc.tensor.matmul(
                            sp[:, 0:512],
                            kt_t[0:64, kt * 128:(kt + 1) * 128],
                            qt_t[0:64, qb * 512:(qb + 1) * 512],
                            start=True, stop=True)
                        nc.tensor.matmul(
                            sp[:, 512:1024],
                            kt_t[64:128, kt * 128:(kt + 1) * 128],
                            qt_t[64:128, qb * 512:(qb + 1) * 512],
                            start=True, stop=True)
                        e = e_sb.tile([128, 1024], F32R, tag="e")
                        nc.scalar.activation(e[:], sp[:], EXP, scale=SCALE)
                        v1v = v1_tiles[kt][:].rearrange("p (s c) -> p s c", c=65)
                        nc.tensor.matmul(ctxA[:], v1v[:, segA, :], e[:, 0:512],
                                         start=(kt == 0), stop=(kt == NKT - 1))
                        nc.tensor.matmul(ctxB[:], v1v[:, segB, :], e[:, 512:1024],
                                         start=(kt == 0), stop=(kt == NKT - 1))
                    for h, ctx in ((0, ctxA), (1, ctxB)):
                        # normalize by 1/Z  (Z in ctx row 64)
                        slot = (2 * p + h) * 4 + qb
                        r1 = r_sb.tile([65, 512], F32, tag="r1")
                        nc.vector.reciprocal(r1[64:65, :], ctx[64:65, :])
                        nc.sync.dma_start(rscr.ap()[slot:slot + 1, :], r1[64:65, :])
                        rb = r_sb.tile([64, 512], F32, tag="rb")
                        nc.gpsimd.dma_start(rb[:], bass.AP(
                            tensor=rscr, offset=slot * 512, ap=[[0, 64], [1, 512]]))
                        if h == 0:
                            nc.vector.tensor_mul(
                                ctxn[p][0:64, qb * 512:(qb + 1) * 512],
                                ctx[0:64, :], rb[:])
                        else:
                            cn = r_sb.tile([64, 512], F32R, tag="cn")
                            nc.vector.tensor_mul(cn[:], ctx[0:64, :], rb[:])
                            # partition shift 0..63 -> 64..127 must go via DMA
                            nc.sync.dma_start(
                                ctxn[p][64:128, qb * 512:(qb + 1) * 512], cn[:])

            # ---- o-projection: y^T[o, q] = Wo^T @ ctxn ----
            wo_t = []
            for ftile in range(4):
                t = w_sb.tile([128, 1024], F32R, tag=f"wo{ftile}")
                nc.sync.dma_start(t[:], wo.ap()[ftile * 128:(ftile + 1) * 128, :].bitcast(F32R))
                wo_t.append(t)
            for ot in range(8):
                for qb in range(NQB):
                    yp = ps_s.tile([128, 512], F32, tag="sp")
                    for ftile in range(4):
                        nc.tensor.matmul(yp[:],
                                         wo_t[ftile][:, ot * 128:(ot + 1) * 128],
                                         ctxn[ftile][:, qb * 512:(qb + 1) * 512],
                                         start=(ftile == 0), stop=(ftile == 3))
                    ys = y_sb.tile([128, 512], F32, tag="ys")
                    nc.vector.tensor_copy(ys[:], yp[:])
                    nc.sync.dma_start(
                        yT.ap()[ot * 128:(ot + 1) * 128, qb * 512:(qb + 1) * 512], ys[:])
    return nc


_nc_cache = None


def _get_nc():
    global _nc_cache
    if _nc_cache is None:
        _install_compile_patch()
        _nc_cache = _build()
    return _nc_cache


def _execute(inputs, trace=False, tmpdir=None):
    from concourse.bass_utils import run_bass_kernel_spmd

    x = np.asarray(inputs["x"], dtype=np.float32)
    Wq = np.asarray(inputs["Wq"], dtype=np.float32)
    Wk = np.asarray(inputs["Wk"], dtype=np.float32)
    Wv = np.asarray(inputs["Wv"], dtype=np.float32)
    Wo = np.asarray(inputs["Wo"], dtype=np.float32)
    bq = np.asarray(inputs["bq"], dtype=np.float32)
    bk = np.asarray(inputs["bk"], dtype=np.float32)
    bv = np.asarray(inputs["bv"], dtype=np.float32)
    bo = np.asarray(inputs["bo"], dtype=np.float32)

    nc = _get_nc()
    in_maps = []
    for c in range(N_CORES):
        b, g = c // 2, c % 2
        sl = slice(g * FT, (g + 1) * FT)
        bv_g = bv[sl].reshape(8, 64)
        bvones = np.concatenate(
            [np.concatenate([bv_g, np.ones((8, 1), np.float32)], axis=1).reshape(-1)])
        in_maps.append({
            "xT": np.ascontiguousarray(x[b].T),
            "wq": np.ascontiguousarray(Wq[:, sl]),
            "wk": np.ascontiguousarray(Wk[:, sl]),
            "wv": np.ascontiguousarray(Wv[:, sl]),
            "wo": np.ascontiguousarray(Wo[sl, :]),
            "bq": np.ascontiguousarray(bq[sl]),
            "bk": np.ascontiguousarray(bk[sl]),
            "bvones": bvones.astype(np.float32),
        })

    kwargs = {}
    if trace:
        kwargs = dict(trace=True, tmpdir=tmpdir)
    res = run_bass_kernel_spmd(nc, in_maps, core_ids=list(range(N_CORES)), **kwargs)

    out = np.empty((B, S, D), dtype=np.float32)
    for b in range(B):
        yT0 = res.results[2 * b]["yT"]
        yT1 = res.results[2 * b + 1]["yT"]
        out[b] = (yT0 + yT1).T + bo
    return out, res


def kernel(**inputs) -> np.ndarray:
    out, _ = _execute(inputs, trace=False)
    return out



# revision 2
# speedup vs baseline: 1.0078x; 1.0078x over previous
"""Multi-head attention (B=4, S=2048, D=1024, H=16, dh=64) on 8 TRN2 NeuronCores.

Sharding: core c = (batch b, head-group g), b = c // 2, g = c % 2.
Each core: 8 heads of one batch. Host sums the per-core / per-pass o-proj
partials and adds bo.

v2 design (vs baseline):
  - All matmul operands bf16 (PSUM accumulation f32): halves DMA + SBUF,
    enables FWL weight loads. Tolerance 2e-2 >> bf16 error.
  - xT persistent in SBUF (loaded once, not per pass).
  - Scores use row-tiled 64x128 PE mode: head A on PE rows 0-63 (tile 0,0),
    head B on rows 64-127 (tile 64,0) -> both stream concurrently.
  - Software pipeline: projections for pass p+1, V for next half, and
    o-projection of pass p-1 run as "filler" tensor work interleaved into
    the attention stream of pass p, so TensorE never idles (keeps p-state
    at 2.4 GHz) while ScalarE (exp, the bottleneck) stays saturated.
  - Z (softmax denom) path: Z row -> DRAM bounce -> stride-0 partition
    broadcast DMA -> reciprocal on [64,512] (not the disastrous
    1-partition reciprocal) -> multiply.
  - o-proj emitted per pass (single matmul per (ot, qb), no cross-pass
    PSUM accumulation); partial [1024,2048] f32 per pass DMA'd straight
    from PSUM to DRAM; host sums the 4 partials per core.
"""
import json
import sys
from collections import deque

sys.path.insert(0, "/opt/trn_rl_repo")

import numpy as np

import concourse.bass as bass
import concourse.tile as tile
from concourse import library_config, mybir

F32 = mybir.dt.float32
BF16 = mybir.dt.bfloat16
EXP = mybir.ActivationFunctionType.Exp

D = 1024          # d_model
S = 2048          # sequence
B = 4             # batch
FT = 512          # features per core (8 heads x 64)
NKT_IN = 8        # k-tiles over d_model
NQB = 4           # 512-query blocks
NKT = 16          # 128-key tiles
NTT = 16          # 128-token tiles
SCALE = 1.0 / 8.0
N_CORES = 8
NPASS = 4         # feature passes (2 heads each)


# --------------------------------------------------------------------------
# walrus in this container rejects instructions with >1 sync wait; split the
# extras onto preceding same-engine NoOps (semantically identical).
def _fix_bir_json(bir_bytes):
    j = json.loads(bir_bytes)
    n = 0
    for fn in j["functions"]:
        for blk in fn["blocks"]:
            out = []
            changed = False
            for inst in blk["instructions"]:
                si = inst.get("sync_info")
                waits = si.get("on_wait") if si else None
                if waits and len(waits) > 1:
                    for w in waits[:-1]:
                        n += 1
                        nop = {
                            "name": f"I-wsplit-{n}",
                            "opcode": "NoOp",
                            "engine": inst["engine"],
                            "ins": [],
                            "outs": [],
                            "sync_info": {"on_wait": [w], "on_update": []},
                        }
                        if "debug" in inst:
                            nop["debug"] = inst["debug"]
                        out.append(nop)
                    si["on_wait"] = [waits[-1]]
                    changed = True
                out.append(inst)
            if changed:
                blk["instructions"] = out
    return json.dumps(j).encode()


def _install_compile_patch():
    import concourse.bass_utils as _bu
    import concourse.bass2jax as _b2j

    if getattr(_bu, "_waitfix_installed", False):
        return
    _orig = _bu.compile_bir_kernel

    def _patched(bir_json, tmpdir, neff_name="file.neff"):
        return _orig(_fix_bir_json(bir_json), tmpdir, neff_name)

    _bu.compile_bir_kernel = _patched
    _b2j.compile_bir_kernel = _patched
    _bu._waitfix_installed = True


# --------------------------------------------------------------------------
def _build():
    nc = bass.Bass("TRN2", target_bir_lowering=False, debug=False,
                   enable_asserts=False, num_devices=N_CORES)

    xT = nc.dram_tensor("xT", [D, S], BF16, kind="ExternalInput")
    wq = nc.dram_tensor("wq", [D, FT], BF16, kind="ExternalInput")
    wk = nc.dram_tensor("wk", [D, FT], BF16, kind="ExternalInput")
    wv = nc.dram_tensor("wv", [D, FT], BF16, kind="ExternalInput")
    wo = nc.dram_tensor("wo", [FT, D], BF16, kind="ExternalInput")
    bq = nc.dram_tensor("bq", [FT], F32, kind="ExternalInput")
    bk = nc.dram_tensor("bk", [FT], F32, kind="ExternalInput")
    # bv interleaved with ones: per head 64 bias + 1.0 -> [8 * 65]
    bvones = nc.dram_tensor("bvones", [8 * 65], F32, kind="ExternalInput")
    yTp = nc.dram_tensor("yTp", [NPASS * D, S], BF16, kind="ExternalOutput")
    zscr = nc.dram_tensor("zscr", [128, 512], F32, kind="Internal")

    with tile.TileContext(nc) as tc:
        with tc.tile_pool(name="x_sb", bufs=1) as x_sb, \
             tc.tile_pool(name="w_sb", bufs=1) as w_sb, \
             tc.tile_pool(name="qk_sb", bufs=2) as qk_sb, \
             tc.tile_pool(name="v1_sb", bufs=2) as v1_sb, \
             tc.tile_pool(name="e_sb", bufs=1) as e_sb, \
             tc.tile_pool(name="ctxn_sb", bufs=1) as ctxn_sb, \
             tc.tile_pool(name="b_sb", bufs=1) as b_sb, \
             tc.tile_pool(name="rz_sb", bufs=2) as rz_sb, \
             tc.tile_pool(name="cn_sb", bufs=2) as cn_sb, \
             tc.tile_pool(name="ps_s", bufs=2, space="PSUM") as ps_s, \
             tc.tile_pool(name="ps_c", bufs=1, space="PSUM") as ps_c, \
             tc.tile_pool(name="ps_p", bufs=2, space="PSUM") as ps_p:

            # ---------------- persistent loads ----------------
            # Need-order: x + pass-0 Q/K weights + half-0 V weights first
            # (split across the sync and gpsimd DMA queues), then the rest.
            xts = []
            for kt in range(NKT_IN):
                t = x_sb.tile([128, S], BF16, tag=f"x{kt}", name=f"x{kt}")
                nc.sync.dma_start(t[:], xT.ap()[kt * 128:(kt + 1) * 128, :])
                xts.append(t)

            wq_t = {}   # (p, kt) -> [128,128]
            wk_t = {}
            wv_t = {}   # (half, kt) -> [128,256]

            def load_wqk(p):
                for kt in range(NKT_IN):
                    tq = w_sb.tile([128, 128], BF16, tag=f"wq{p}_{kt}", name=f"wq{p}_{kt}")
                    nc.sync.dma_start(tq[:], wq.ap()[kt * 128:(kt + 1) * 128,
                                                     p * 128:(p + 1) * 128])
                    wq_t[(p, kt)] = tq
                    tk = w_sb.tile([128, 128], BF16, tag=f"wk{p}_{kt}", name=f"wk{p}_{kt}")
                    nc.sync.dma_start(tk[:], wk.ap()[kt * 128:(kt + 1) * 128,
                                                     p * 128:(p + 1) * 128])
                    wk_t[(p, kt)] = tk

            def load_wv(half):
                for kt in range(NKT_IN):
                    t = w_sb.tile([128, 256], BF16, tag=f"wv{half}_{kt}", name=f"wv{half}_{kt}")
                    nc.gpsimd.dma_start(t[:], wv.ap()[kt * 128:(kt + 1) * 128,
                                                      half * 256:(half + 1) * 256])
                    wv_t[(half, kt)] = t

            load_wqk(0)
            load_wv(0)
            bb_t = []
            for half in range(2):
                t = b_sb.tile([128, 260], F32, tag=f"bb{half}", name=f"bb{half}")
                nc.gpsimd.dma_start(t[:], bass.AP(
                    tensor=bvones, offset=half * 260, ap=[[0, 128], [1, 260]]))
                bb_t.append(t)
            # biases: bq/bk as [128, 4] (partition i, col p = bias[p*128+i])
            bq_t = b_sb.tile([128, NPASS], F32, tag="bq", name="bq_t")
            nc.sync.dma_start(bq_t[:], bass.AP(tensor=bq, offset=0,
                                               ap=[[1, 128], [128, NPASS]]))
            bk_t = b_sb.tile([128, NPASS], F32, tag="bk", name="bk_t")
            nc.sync.dma_start(bk_t[:], bass.AP(tensor=bk, offset=0,
                                               ap=[[1, 128], [128, NPASS]]))
            # the rest trickles in behind (gpsimd queue)
            load_wv(1)
            for p in range(1, NPASS):
                load_wqk(p)
            wo_t = []
            for p in range(NPASS):
                t = w_sb.tile([128, D], BF16, tag=f"wo{p}", name=f"wo{p}")
                nc.gpsimd.dma_start(t[:], wo.ap()[p * 128:(p + 1) * 128, :])
                wo_t.append(t)

            # ---------------- persistent compute tiles ----------------
            ctxn = [ctxn_sb.tile([128, S], BF16, tag=f"ctxn{p}", name=f"ctxn{p}")
                    for p in range(NPASS)]
            qt_cur = {}   # p -> tile  (created lazily, bufs=2 round robin)
            kt_cur = {}
            v1_cur = {}   # half -> list of 16 tiles

            # ---------------- emission helpers ----------------
            def emit_qkproj(p, which, qbp):
                """One 8-matmul accum group computing qt/kt for pass p,
                query-block qbp, plus the bias add."""
                if which == "q":
                    if p not in qt_cur:
                        qt_cur[p] = qk_sb.tile([128, S], BF16, tag="qt", name=f"qt{p}")
                    dst, wt, bt = qt_cur[p], wq_t, bq_t
                else:
                    if p not in kt_cur:
                        kt_cur[p] = qk_sb.tile([128, S], BF16, tag="kt", name=f"kt{p}")
                    dst, wt, bt = kt_cur[p], wk_t, bk_t
                pp = ps_p.tile([128, 512], F32, tag="pp", name="pp")
                for kt in range(NKT_IN):
                    nc.tensor.matmul(pp[:], wt[(p, kt)][:],
                                     xts[kt][:, qbp * 512:(qbp + 1) * 512],
                                     start=(kt == 0), stop=(kt == NKT_IN - 1))
                nc.vector.tensor_scalar_add(
                    dst[:, qbp * 512:(qbp + 1) * 512], pp[:], bt[:, p:p + 1])

            def emit_vproj(half, tt):
                """V tile tt for 4 heads of `half` + bias + ones column."""
                if half not in v1_cur:
                    v1_cur[half] = {}
                pv = ps_p.tile([128, 512], F32, tag="pp", name="pp")
                for kt in range(NKT_IN):
                    nc.tensor.matmul(pv[:, 0:256], xts[kt][:, tt * 128:(tt + 1) * 128],
                                     wv_t[(half, kt)][:],
                                     start=(kt == 0), stop=(kt == NKT_IN - 1))
                vt = v1_sb.tile([128, 260], BF16, tag=f"v1_{tt}", name=f"v1_{tt}")
                v1_cur[half][tt] = vt
                v1v = vt[:].rearrange("p (s c) -> p s c", c=65)
                nc.vector.tensor_add(
                    v1v[:, :, 0:64],
                    pv[:, 0:256].rearrange("p (s c) -> p s c", c=64),
                    bb_t[half][:].rearrange("p (s c) -> p s c", c=65)[:, :, 0:64])
                nc.gpsimd.memset(v1v[:, :, 64:65], 1.0)

            def emit_oproj(p, ot, qb, evac="v"):
                yp = ps_p.tile([128, 512], F32, tag="pp", name="pp")
                nc.tensor.matmul(yp[:], wo_t[p][:, ot * 128:(ot + 1) * 128],
                                 ctxn[p][:, qb * 512:(qb + 1) * 512],
                                 start=True, stop=True)
                ys = cn_sb.tile([128, 512], BF16, tag="ys", name="ys", bufs=4)
                if evac == "s":
                    nc.scalar.copy(ys[:], yp[:])
                else:
                    nc.vector.tensor_copy(ys[:], yp[:])
                nc.sync.dma_start(
                    yTp.ap()[p * D + ot * 128: p * D + (ot + 1) * 128,
                             qb * 512:(qb + 1) * 512], ys[:])

            # filler machinery: (est_tensor_ns, closure)
            filler = deque()
            # two-stage deferred normalize (each stage runs 1 qb after its
            # inputs were produced, so the strictly-FIFO DVE queue never
            # head-of-line blocks on cross-queue DMA latency)
            pending_recip = deque()
            pending_mul = deque()

            def drain(budget):
                spent = 0
                while filler and (spent == 0 or spent + filler[0][0] <= budget):
                    cost, fn = filler.popleft()
                    fn()
                    spent += cost
                return spent

            def mk(fn, *a):
                return lambda: fn(*a)

            def ensure(cond_fn):
                while filler and not cond_fn():
                    filler.popleft()[1]()

            # ---------------- prologue ----------------
            # pass-0 Q/K for qb'=0 and V depends are emitted directly; the
            # rest rides the filler queue at elevated budget during pass 0.
            emit_qkproj(0, "q", 0)
            emit_qkproj(0, "k", 0)
            for qbp in range(1, NQB):
                filler.append((1700, mk(emit_qkproj, 0, "k", qbp)))
            for tt in range(4):
                filler.append((900, mk(emit_vproj, 0, tt)))
            for qbp in range(1, NQB):
                filler.append((1700, mk(emit_qkproj, 0, "q", qbp)))
            for tt in range(4, NTT):
                filler.append((900, mk(emit_vproj, 0, tt)))

            # ---------------- main passes ----------------
            for p in range(NPASS):
                half = p // 2
                segA = (2 * p) % 4
                segB = (2 * p + 1) % 4

                # enqueue this pass's background work
                if p < NPASS - 1:
                    for qbp in range(NQB):
                        filler.append((1700, mk(emit_qkproj, p + 1, "q", qbp)))
                        filler.append((1700, mk(emit_qkproj, p + 1, "k", qbp)))
                if p == 1:
                    for tt in range(NTT):
                        filler.append((900, mk(emit_vproj, 1, tt)))


                ensure(lambda: p in qt_cur and p in kt_cur)
                qt_t = qt_cur[p]
                kt_t = kt_cur[p]

                for qb in range(NQB):
                    # flush deferred normalize stages: qb-1's recip first
                    # (inputs always ready -> never head-blocks the DVE
                    # FIFO), then qb-2's muls (their broadcasts landed a
                    # full qb ago)
                    stage2 = [pending_recip.popleft()() for _ in range(len(pending_recip))]
                    while pending_mul:
                        pending_mul.popleft()()
                    pending_mul.extend(stage2)
                    if qb == 1 and p >= 1:
                        # o-proj of the previous pass, qb-major so the
                        # freshest ctxn slice (qb3) drains last
                        for qbo in range(NQB):
                            for ot in range(8):
                                filler.append((700, mk(emit_oproj, p - 1, ot, qbo)))
                    e_tiles = {}
                    sp_tiles = {}

                    def emit_scores(kt):
                        sp = ps_s.tile([128, 1024], F32, tag="sp", name="sp")
                        sp_tiles[kt] = sp
                        nc.tensor.matmul(
                            sp[:, 0:512],
                            kt_t[0:64, kt * 128:(kt + 1) * 128],
                            qt_t[0:64, qb * 512:(qb + 1) * 512],
                            start=True, stop=True)
                        nc.tensor.matmul(
                            sp[:, 512:1024],
                            kt_t[64:128, kt * 128:(kt + 1) * 128],
                            qt_t[64:128, qb * 512:(qb + 1) * 512],
                            start=True, stop=True)
                        e = e_sb.tile([128, 1024], BF16, tag=f"e{kt}", name=f"e{kt}")
                        e_tiles[kt] = e
                        nc.scalar.activation(e[:], sp[:], EXP, scale=SCALE)

                    ctxA = ps_c.tile([65, 512], F32, tag="ctxA", name="ctxA")
                    ctxB = ps_c.tile([65, 512], F32, tag="ctxB", name="ctxB")

                    def emit_pv(kt):
                        ensure(lambda: kt in v1_cur.get(half, {}))
                        v1v = v1_cur[half][kt][:].rearrange("p (s c) -> p s c", c=65)
                        e = e_tiles[kt]
                        nc.tensor.matmul(ctxA[:], v1v[:, segA, :], e[:, 0:512],
                                         start=(kt == 0), stop=(kt == NKT - 1),
                                         skip_group_check=True)
                        nc.tensor.matmul(ctxB[:], v1v[:, segB, :], e[:, 512:1024],
                                         start=(kt == 0), stop=(kt == NKT - 1),
                                         skip_group_check=True)

                    # adaptive filler budget for this qb's 8 slots
                    slots_left_total = (NQB - qb) * 8 + (NQB * 8) * (NPASS - 1 - p)
                    pend = sum(c for c, _ in filler)
                    budget = max(500, min(2500, pend // max(slots_left_total, 1)))

                    for g in range(8):
                        emit_scores(2 * g)
                        emit_scores(2 * g + 1)
                        if g >= 1:
                            emit_pv(2 * (g - 1))
                            emit_pv(2 * (g - 1) + 1)
                        drain(budget)
                    emit_pv(14)
                    drain(budget)
                    emit_pv(15)
                    drain(budget)

                    # ---- normalize (3-stage, each stage one qb apart) ----
                    # Stage 0 (now): release ctx PSUM via raw f32 copies to
                    # SBUF (row 64 carries Z); ship both Z rows to DRAM.
                    # Stage 1 (next qb): reload Z partition-packed [128,8]
                    # (reciprocal there costs ~8 free elems, not 512),
                    # store back, stride-0 broadcast to [64,512].
                    # Stage 2 (qb after): multiply + head-B partition shift.
                    # All bounce DMAs ride the sync queue in FIFO order.
                    slotA = (p * NQB + qb) * 2
                    slotB = slotA + 1
                    csA = rz_sb.tile([65, 512], F32, tag="csA", name="csA", bufs=3)
                    nc.vector.tensor_copy(csA[:], ctxA[:])
                    csB = rz_sb.tile([65, 512], F32, tag="csB", name="csB", bufs=3)
                    nc.vector.tensor_copy(csB[:], ctxB[:])
                    nc.sync.dma_start(zscr.ap()[slotA:slotA + 1, :], csA[64:65, :])
                    nc.sync.dma_start(zscr.ap()[slotB:slotB + 1, :], csB[64:65, :])
                    # partition-packed Z reload: 64 partitions x 16 contiguous
                    # f32 (64B/descriptor; the naive transpose AP would be
                    # 1024 4-byte descriptors and ~11us of DMA latency)
                    zsm = rz_sb.tile([64, 16], F32, tag="zsm", name="zsm", bufs=3)
                    nc.gpsimd.dma_start(zsm[:], bass.AP(
                        tensor=zscr, offset=slotA * 512, ap=[[16, 64], [1, 16]]))

                    def stage_recip(p=p, qb=qb, slotA=slotA, slotB=slotB,
                                    csA=csA, csB=csB, zsm=zsm):
                        zsm2 = rz_sb.tile([64, 16], F32, tag="zsm2", name="zsm2")
                        nc.vector.reciprocal(zsm2[:], zsm[:])
                        nc.sync.dma_start(bass.AP(
                            tensor=zscr, offset=(64 + slotA) * 512,
                            ap=[[16, 64], [1, 16]]), zsm2[:])
                        rzA = rz_sb.tile([64, 512], F32, tag="rzA", name="rzA")
                        nc.sync.dma_start(rzA[:], bass.AP(
                            tensor=zscr, offset=(64 + slotA) * 512, ap=[[0, 64], [1, 512]]))
                        rzB = rz_sb.tile([64, 512], F32, tag="rzB", name="rzB")
                        nc.sync.dma_start(rzB[:], bass.AP(
                            tensor=zscr, offset=(64 + slotB) * 512, ap=[[0, 64], [1, 512]]))

                        def stage_mul():
                            nc.vector.tensor_mul(
                                ctxn[p][0:64, qb * 512:(qb + 1) * 512],
                                csA[0:64, :], rzA[:])
                            cn = cn_sb.tile([64, 512], BF16, tag="cn", name="cn")
                            nc.vector.tensor_mul(cn[:], csB[0:64, :], rzB[:])
                            nc.sync.dma_start(
                                ctxn[p][64:128, qb * 512:(qb + 1) * 512], cn[:])
                            if p == 3:
                                for ot in range(8):
                                    ev = "s" if (qb >= 2 and ot % 2 == 0) else "v"
                                    filler.append((700, mk(emit_oproj, 3, ot, qb, ev)))
                        return stage_mul
                    pending_recip.append(stage_recip)

            # tail: flush deferred normalize stages, then remaining work
            while pending_mul:
                pending_mul.popleft()()
            while pending_recip:
                pending_mul.append(pending_recip.popleft()())
            while pending_mul:
                pending_mul.popleft()()
            while filler:
                filler.popleft()[1]()
    return nc


_nc_cache = None


def _get_nc():
    global _nc_cache
    if _nc_cache is None:
        _install_compile_patch()
        _nc_cache = _build()
    return _nc_cache


def _execute(inputs, trace=False, tmpdir=None):
    from concourse.bass_utils import run_bass_kernel_spmd

    bf = mybir.dt.np(BF16)
    x = np.asarray(inputs["x"], dtype=np.float32)
    Wq = np.asarray(inputs["Wq"], dtype=np.float32)
    Wk = np.asarray(inputs["Wk"], dtype=np.float32)
    Wv = np.asarray(inputs["Wv"], dtype=np.float32)
    Wo = np.asarray(inputs["Wo"], dtype=np.float32)
    bq = np.asarray(inputs["bq"], dtype=np.float32)
    bk = np.asarray(inputs["bk"], dtype=np.float32)
    bv = np.asarray(inputs["bv"], dtype=np.float32)
    bo = np.asarray(inputs["bo"], dtype=np.float32)

    nc = _get_nc()
    in_maps = []
    for c in range(N_CORES):
        b, g = c // 2, c % 2
        sl = slice(g * FT, (g + 1) * FT)
        bv_g = bv[sl].reshape(8, 64)
        bvones = np.concatenate(
            [bv_g, np.ones((8, 1), np.float32)], axis=1).reshape(-1)
        in_maps.append({
            "xT": np.ascontiguousarray(x[b].T).astype(bf),
            "wq": np.ascontiguousarray(Wq[:, sl]).astype(bf),
            "wk": np.ascontiguousarray(Wk[:, sl]).astype(bf),
            "wv": np.ascontiguousarray(Wv[:, sl]).astype(bf),
            "wo": np.ascontiguousarray(Wo[sl, :]).astype(bf),
            "bq": np.ascontiguousarray(bq[sl]),
            "bk": np.ascontiguousarray(bk[sl]),
            "bvones": bvones.astype(np.float32),
        })

    kwargs = {}
    if trace:
        kwargs = dict(trace=True, tmpdir=tmpdir)
    res = run_bass_kernel_spmd(nc, in_maps, core_ids=list(range(N_CORES)), **kwargs)

    out = np.empty((B, S, D), dtype=np.float32)
    for b in range(B):
        a0 = res.results[2 * b]["yTp"].reshape(NPASS, D, S).sum(axis=0)
        a1 = res.results[2 * b + 1]["yTp"].reshape(NPASS, D, S).sum(axis=0)
        out[b] = (a0 + a1).T + bo
    return out, res


def kernel(**inputs) -> np.ndarray:
    out, _ = _execute(inputs, trace=False)
    return out


# revision 4
# speedup vs baseline: 1.0479x; 1.0398x over previous
"""Multi-head attention (B=4, S=2048, D=1024, H=16, dh=64) on 8 TRN2 NeuronCores.

Sharding: core c = (batch b, head-group g), b = c // 2, g = c % 2.
Each core: 8 heads of one batch. Host sums the per-core / per-pass o-proj
partials and adds bo.

v2 design (vs baseline):
  - All matmul operands bf16 (PSUM accumulation f32): halves DMA + SBUF,
    enables FWL weight loads. Tolerance 2e-2 >> bf16 error.
  - xT persistent in SBUF (loaded once, not per pass).
  - Scores use row-tiled 64x128 PE mode: head A on PE rows 0-63 (tile 0,0),
    head B on rows 64-127 (tile 64,0) -> both stream concurrently.
  - Software pipeline: projections for pass p+1, V for next half, and
    o-projection of pass p-1 run as "filler" tensor work interleaved into
    the attention stream of pass p, so TensorE never idles (keeps p-state
    at 2.4 GHz) while ScalarE (exp, the bottleneck) stays saturated.
  - Z (softmax denom) path: Z row -> DRAM bounce -> stride-0 partition
    broadcast DMA -> reciprocal on [64,512] (not the disastrous
    1-partition reciprocal) -> multiply.
  - o-proj emitted per pass (single matmul per (ot, qb), no cross-pass
    PSUM accumulation); partial [1024,2048] f32 per pass DMA'd straight
    from PSUM to DRAM; host sums the 4 partials per core.
"""
import json
import sys
from collections import deque

sys.path.insert(0, "/opt/trn_rl_repo")

import numpy as np

import concourse.bass as bass
import concourse.tile as tile
from concourse import library_config, mybir

F32 = mybir.dt.float32
BF16 = mybir.dt.bfloat16
EXP = mybir.ActivationFunctionType.Exp

D = 1024          # d_model
S = 2048          # sequence
B = 4             # batch
FT = 512          # features per core (8 heads x 64)
NKT_IN = 8        # k-tiles over d_model
NQB = 4           # 512-query blocks
NKT = 16          # 128-key tiles
NTT = 16          # 128-token tiles
SCALE = 1.0 / 8.0
N_CORES = 8
NPASS = 4         # feature passes (2 heads each)


# --------------------------------------------------------------------------
# walrus in this container rejects instructions with >1 sync wait; split the
# extras onto preceding same-engine NoOps (semantically identical).
def _fix_bir_json(bir_bytes):
    j = json.loads(bir_bytes)
    n = 0
    for fn in j["functions"]:
        for blk in fn["blocks"]:
            out = []
            changed = False
            for inst in blk["instructions"]:
                si = inst.get("sync_info")
                waits = si.get("on_wait") if si else None
                if waits and len(waits) > 1:
                    for w in waits[:-1]:
                        n += 1
                        nop = {
                            "name": f"I-wsplit-{n}",
                            "opcode": "NoOp",
                            "engine": inst["engine"],
                            "ins": [],
                            "outs": [],
                            "sync_info": {"on_wait": [w], "on_update": []},
                        }
                        if "debug" in inst:
                            nop["debug"] = inst["debug"]
                        out.append(nop)
                    si["on_wait"] = [waits[-1]]
                    changed = True
                out.append(inst)
            if changed:
                blk["instructions"] = out
    return json.dumps(j).encode()


def _install_compile_patch():
    import concourse.bass_utils as _bu
    import concourse.bass2jax as _b2j

    if getattr(_bu, "_waitfix_installed", False):
        return
    _orig = _bu.compile_bir_kernel

    def _patched(bir_json, tmpdir, neff_name="file.neff"):
        return _orig(_fix_bir_json(bir_json), tmpdir, neff_name)

    _bu.compile_bir_kernel = _patched
    _b2j.compile_bir_kernel = _patched
    _bu._waitfix_installed = True


# --------------------------------------------------------------------------
def _build():
    nc = bass.Bass("TRN2", target_bir_lowering=False, debug=False,
                   enable_asserts=False, num_devices=N_CORES)

    xT = nc.dram_tensor("xT", [D, S], BF16, kind="ExternalInput")
    # wq/wk: host pre-tiled [128, pass*1024 + kt*128 + j] so each pass's
    # 8 lhsT tiles load as ONE contiguous-per-partition DMA (128
    # descriptors instead of 8 separate ~600ns DMA issues)
    wq = nc.dram_tensor("wq", [128, 4 * 1024], BF16, kind="ExternalInput")
    wk = nc.dram_tensor("wk", [128, 4 * 1024], BF16, kind="ExternalInput")
    # wv: host pre-tiled [128, half*2048 + kt*256 + j]
    wv = nc.dram_tensor("wv", [128, 2 * 2048], BF16, kind="ExternalInput")
    wo = nc.dram_tensor("wo", [FT, D], BF16, kind="ExternalInput")
    bq = nc.dram_tensor("bq", [FT], F32, kind="ExternalInput")
    bk = nc.dram_tensor("bk", [FT], F32, kind="ExternalInput")
    # bv interleaved with ones: per head 64 bias + 1.0 -> [8 * 65]
    bvones = nc.dram_tensor("bvones", [8 * 65], F32, kind="ExternalInput")
    yTp = nc.dram_tensor("yTp", [NPASS * D, S], BF16, kind="ExternalOutput")
    zscr = nc.dram_tensor("zscr", [128, 512], F32, kind="Internal")

    with tile.TileContext(nc) as tc:
        with tc.tile_pool(name="x_sb", bufs=1) as x_sb, \
             tc.tile_pool(name="w_sb", bufs=1) as w_sb, \
             tc.tile_pool(name="qk_sb", bufs=2) as qk_sb, \
             tc.tile_pool(name="v1_sb", bufs=2) as v1_sb, \
             tc.tile_pool(name="e_sb", bufs=1) as e_sb, \
             tc.tile_pool(name="ctxn_sb", bufs=1) as ctxn_sb, \
             tc.tile_pool(name="b_sb", bufs=1) as b_sb, \
             tc.tile_pool(name="rz_sb", bufs=2) as rz_sb, \
             tc.tile_pool(name="cn_sb", bufs=2) as cn_sb, \
             tc.tile_pool(name="ps_s", bufs=2, space="PSUM") as ps_s, \
             tc.tile_pool(name="ps_c", bufs=1, space="PSUM") as ps_c, \
             tc.tile_pool(name="ps_p", bufs=2, space="PSUM") as ps_p:

            # ---------------- persistent loads ----------------
            # Need-order: pass-0 Q/K weights (one fused DMA each), then x
            # split across both DMA queues; half-0 V weights and biases on
            # the gpsimd queue in parallel.
            wq_p = {}   # p -> [128,1024]; kt slice = [:, kt*128:(kt+1)*128]
            wk_p = {}
            wv_h = {}   # half -> [128,2048]; kt slice = [:, kt*256:...]

            def load_wqk(p):
                tq = w_sb.tile([128, 1024], BF16, tag=f"wq{p}", name=f"wq{p}")
                nc.sync.dma_start(tq[:], wq.ap()[:, p * 1024:(p + 1) * 1024])
                wq_p[p] = tq
                tk = w_sb.tile([128, 1024], BF16, tag=f"wk{p}", name=f"wk{p}")
                nc.sync.dma_start(tk[:], wk.ap()[:, p * 1024:(p + 1) * 1024])
                wk_p[p] = tk

            def load_wv(half):
                t = w_sb.tile([128, 2048], BF16, tag=f"wv{half}", name=f"wv{half}")
                nc.gpsimd.dma_start(t[:], wv.ap()[:, half * 2048:(half + 1) * 2048])
                wv_h[half] = t

            load_wqk(0)
            load_wv(0)
            xts = []
            for kt in range(NKT_IN):
                t = x_sb.tile([128, S], BF16, tag=f"x{kt}", name=f"x{kt}")
                eng = nc.sync if kt % 2 == 0 else nc.gpsimd
                eng.dma_start(t[:], xT.ap()[kt * 128:(kt + 1) * 128, :])
                xts.append(t)
            bb_t = []
            for half in range(2):
                t = b_sb.tile([128, 260], F32, tag=f"bb{half}", name=f"bb{half}")
                nc.gpsimd.dma_start(t[:], bass.AP(
                    tensor=bvones, offset=half * 260, ap=[[0, 128], [1, 260]]))
                bb_t.append(t)
            # biases: bq/bk as [128, 4] (partition i, col p = bias[p*128+i])
            bq_t = b_sb.tile([128, NPASS], F32, tag="bq", name="bq_t")
            nc.sync.dma_start(bq_t[:], bass.AP(tensor=bq, offset=0,
                                               ap=[[1, 128], [128, NPASS]]))
            bk_t = b_sb.tile([128, NPASS], F32, tag="bk", name="bk_t")
            nc.sync.dma_start(bk_t[:], bass.AP(tensor=bk, offset=0,
                                               ap=[[1, 128], [128, NPASS]]))
            # the rest trickles in behind (gpsimd queue)
            load_wv(1)
            for p in range(1, NPASS):
                load_wqk(p)
            wo_t = []
            for p in range(NPASS):
                t = w_sb.tile([128, D], BF16, tag=f"wo{p}", name=f"wo{p}")
                nc.gpsimd.dma_start(t[:], wo.ap()[p * 128:(p + 1) * 128, :])
                wo_t.append(t)

            # ---------------- persistent compute tiles ----------------
            ctxn = [ctxn_sb.tile([128, S], BF16, tag=f"ctxn{p}", name=f"ctxn{p}")
                    for p in range(NPASS)]
            qt_cur = {}   # p -> tile  (created lazily, bufs=2 round robin)
            kt_cur = {}
            v1_cur = {}   # half -> list of 16 tiles

            # ---------------- emission helpers ----------------
            qk_done = set()   # (p, which, qbp) proj groups fully emitted

            def emit_qkproj(p, which, qbp, part, state):
                """Half of an 8-matmul accum group computing qt/kt for pass
                p, query-block qbp (split in two so filler slots stay
                small); part 1 adds the bias."""
                if which == "q":
                    if p not in qt_cur:
                        qt_cur[p] = qk_sb.tile([128, S], BF16, tag="qt", name=f"qt{p}")
                    dst, wt, bt = qt_cur[p], wq_p[p], bq_t
                else:
                    if p not in kt_cur:
                        kt_cur[p] = qk_sb.tile([128, S], BF16, tag="kt", name=f"kt{p}")
                    dst, wt, bt = kt_cur[p], wk_p[p], bk_t
                if part == 0:
                    state["pp"] = ps_p.tile([128, 512], F32, tag="pp", name="pp")
                pp = state["pp"]
                for kt in range(4 * part, 4 * part + 4):
                    nc.tensor.matmul(pp[:], wt[:, kt * 128:(kt + 1) * 128],
                                     xts[kt][:, qbp * 512:(qbp + 1) * 512],
                                     start=(kt == 0), stop=(kt == NKT_IN - 1))
                if part == 1:
                    nc.vector.tensor_scalar_add(
                        dst[:, qbp * 512:(qbp + 1) * 512], pp[:], bt[:, p:p + 1])
                    qk_done.add((p, which, qbp))

            def emit_vproj(half, tt):
                """V tile tt for 4 heads of `half` + bias + ones column."""
                if half not in v1_cur:
                    v1_cur[half] = {}
                pv = ps_p.tile([128, 512], F32, tag="pp", name="pp")
                for kt in range(NKT_IN):
                    nc.tensor.matmul(pv[:, 0:256],
                                     xts[kt][:, tt * 128:(tt + 1) * 128],
                                     wv_h[half][:, kt * 256:(kt + 1) * 256],
                                     start=(kt == 0), stop=(kt == NKT_IN - 1))
                vt = v1_sb.tile([128, 260], BF16, tag=f"v1_{tt}", name=f"v1_{tt}")
                v1_cur[half][tt] = vt
                v1v = vt[:].rearrange("p (s c) -> p s c", c=65)
                nc.vector.tensor_add(
                    v1v[:, :, 0:64],
                    pv[:, 0:256].rearrange("p (s c) -> p s c", c=64),
                    bb_t[half][:].rearrange("p (s c) -> p s c", c=65)[:, :, 0:64])
                nc.gpsimd.memset(v1v[:, :, 64:65], 1.0)

            norm_done = set()   # (p, qb) whose ctxn slice is fully emitted

            def emit_oproj(p, ot, qb, evac="v"):
                yp = ps_p.tile([128, 512], F32, tag="pp", name="pp")
                nc.tensor.matmul(yp[:], wo_t[p][:, ot * 128:(ot + 1) * 128],
                                 ctxn[p][:, qb * 512:(qb + 1) * 512],
                                 start=True, stop=True)
                ys = cn_sb.tile([128, 512], BF16, tag="ys", name="ys", bufs=4)
                if evac == "s":
                    nc.scalar.copy(ys[:], yp[:])
                else:
                    nc.vector.tensor_copy(ys[:], yp[:])
                nc.sync.dma_start(
                    yTp.ap()[p * D + ot * 128: p * D + (ot + 1) * 128,
                             qb * 512:(qb + 1) * 512], ys[:])

            # filler machinery: (est_tensor_ns, closure)
            filler = deque()
            # globally-lagged PV pipeline: one closure per 2-kt group,
            # emitted 3 group-slots after its scores (crosses qb/pass
            # boundaries, so each qb's first scores are never queued
            # behind stale PV work)
            pending_pv = deque()
            # staged deferred normalize (each stage runs >=1 qb after its
            # inputs were produced, so the strictly-FIFO DVE queue never
            # head-of-line blocks on cross-queue DMA latency)
            pending_recip = deque()
            pending_mul = deque()

            def drain(budget):
                # entries: (est_ns, fn) or (est_ns, fn, ready_fn); a
                # not-ready head stops the drain (FIFO order is the
                # correctness order — never skip ahead)
                spent = 0
                while filler and (spent == 0 or spent + filler[0][0] <= budget):
                    ent = filler[0]
                    if len(ent) == 3 and not ent[2]():
                        break
                    filler.popleft()
                    ent[1]()
                    spent += ent[0]
                return spent

            def mk(fn, *a):
                return lambda: fn(*a)

            def ensure(cond_fn):
                while filler and not cond_fn():
                    ent = filler[0]
                    if len(ent) == 3 and not ent[2]():
                        raise RuntimeError("ensure blocked on not-ready filler")
                    filler.popleft()
                    ent[1]()

            # ---------------- prologue ----------------
            # pass-0 Q/K for qb'=0 and V depends are emitted directly; the
            # rest rides the filler queue, interleaved per query block so
            # the qb-start barriers rarely force-drain.
            emit_qkproj(0, "q", 0, 0, st0 := {})
            emit_qkproj(0, "q", 0, 1, st0)
            emit_qkproj(0, "k", 0, 0, st1 := {})
            emit_qkproj(0, "k", 0, 1, st1)
            for qbp in range(1, NQB):
                for which in ("k", "q"):
                    st = {}
                    filler.append((900, mk(emit_qkproj, 0, which, qbp, 0, st)))
                    filler.append((900, mk(emit_qkproj, 0, which, qbp, 1, st)))
                for tt in range(4 * (qbp - 1), 4 * qbp):
                    filler.append((900, mk(emit_vproj, 0, tt)))
            for tt in range(12, NTT):
                filler.append((900, mk(emit_vproj, 0, tt)))

            # ---------------- main passes ----------------
            for p in range(NPASS):
                half = p // 2
                segA = (2 * p) % 4
                segB = (2 * p + 1) % 4

                # enqueue this pass's background work
                if p < NPASS - 1:
                    for qbp in range(NQB):
                        stq, stk = {}, {}
                        filler.append((900, mk(emit_qkproj, p + 1, "q", qbp, 0, stq)))
                        filler.append((900, mk(emit_qkproj, p + 1, "q", qbp, 1, stq)))
                        filler.append((900, mk(emit_qkproj, p + 1, "k", qbp, 0, stk)))
                        filler.append((900, mk(emit_qkproj, p + 1, "k", qbp, 1, stk)))
                if p == 1:
                    for tt in range(NTT):
                        filler.append((900, mk(emit_vproj, 1, tt)))


                ensure(lambda: p in qt_cur and p in kt_cur)
                qt_t = qt_cur[p]
                kt_t = kt_cur[p]

                for qb in range(NQB):
                    # flush deferred normalize stages: recips first (inputs
                    # always ready -> never head-blocks the DVE FIFO), then
                    # older muls (their broadcasts landed long ago)
                    stage2 = [pending_recip.popleft()() for _ in range(len(pending_recip))]
                    while pending_mul:
                        pending_mul.popleft()()
                    pending_mul.extend(stage2)
                    if qb == 1 and p >= 1:
                        # o-proj of the previous pass, qb-major so the
                        # freshest ctxn slice (qb3) drains last; gated on
                        # its normalize having been emitted
                        for qbo in range(NQB):
                            for ot in range(8):
                                filler.append((350, mk(emit_oproj, p - 1, ot, qbo),
                                               (lambda pp_=p - 1, q_=qbo:
                                                (pp_, q_) in norm_done)))
                    e_tiles = {}
                    cq = {}

                    def emit_scores(kt, qb=qb, qt_t=qt_t, kt_t=kt_t, e_tiles=e_tiles):
                        sp = ps_s.tile([128, 1024], F32, tag="sp", name="sp")
                        nc.tensor.matmul(
                            sp[:, 0:512],
                            kt_t[0:64, kt * 128:(kt + 1) * 128],
                            qt_t[0:64, qb * 512:(qb + 1) * 512],
                            start=True, stop=True)
                        nc.tensor.matmul(
                            sp[:, 512:1024],
                            kt_t[64:128, kt * 128:(kt + 1) * 128],
                            qt_t[64:128, qb * 512:(qb + 1) * 512],
                            start=True, stop=True)
                        e = e_sb.tile([128, 1024], BF16, tag=f"e{kt}", name=f"e{kt}")
                        e_tiles[kt] = e
                        nc.scalar.activation(e[:], sp[:], EXP, scale=SCALE)

                    def stage0_norm(p=p, qb=qb, cq=cq):
                        # release ctx PSUM via raw f32 copies (row 64 = Z);
                        # ship Z rows to DRAM; start partition-packed reload
                        slotA = (p * NQB + qb) * 2
                        slotB = slotA + 1
                        csA = rz_sb.tile([65, 512], F32, tag="csA", name="csA", bufs=4)
                        nc.vector.tensor_copy(csA[:], cq["A"][:])
                        csB = rz_sb.tile([65, 512], F32, tag="csB", name="csB", bufs=4)
                        nc.vector.tensor_copy(csB[:], cq["B"][:])
                        nc.sync.dma_start(zscr.ap()[slotA:slotA + 1, :], csA[64:65, :])
                        nc.sync.dma_start(zscr.ap()[slotB:slotB + 1, :], csB[64:65, :])
                        # 64 partitions x 16 contiguous f32 (64B/descriptor;
                        # a [128,8] transpose AP would be 1024 4-byte
                        # descriptors and ~11us of DMA latency)
                        zsm = rz_sb.tile([64, 16], F32, tag="zsm", name="zsm", bufs=4)
                        nc.gpsimd.dma_start(zsm[:], bass.AP(
                            tensor=zscr, offset=slotA * 512, ap=[[16, 64], [1, 16]]))

                        def stage_recip():
                            zsm2 = rz_sb.tile([64, 16], F32, tag="zsm2", name="zsm2")
                            nc.vector.reciprocal(zsm2[:], zsm[:])
                            nc.sync.dma_start(bass.AP(
                                tensor=zscr, offset=(64 + slotA) * 512,
                                ap=[[16, 64], [1, 16]]), zsm2[:])
                            rzA = rz_sb.tile([64, 512], F32, tag="rzA", name="rzA", bufs=3)
                            nc.sync.dma_start(rzA[:], bass.AP(
                                tensor=zscr, offset=(64 + slotA) * 512, ap=[[0, 64], [1, 512]]))
                            rzB = rz_sb.tile([64, 512], F32, tag="rzB", name="rzB", bufs=3)
                            nc.sync.dma_start(rzB[:], bass.AP(
                                tensor=zscr, offset=(64 + slotB) * 512, ap=[[0, 64], [1, 512]]))

                            def stage_mul():
                                nc.vector.tensor_mul(
                                    ctxn[p][0:64, qb * 512:(qb + 1) * 512],
                                    csA[0:64, :], rzA[:])
                                cn = cn_sb.tile([64, 512], BF16, tag="cn", name="cn")
                                nc.vector.tensor_mul(cn[:], csB[0:64, :], rzB[:])
                                nc.sync.dma_start(
                                    ctxn[p][64:128, qb * 512:(qb + 1) * 512], cn[:])
                                norm_done.add((p, qb))
                                if p == 3:
                                    for ot in range(8):
                                        ev = "s" if (qb >= 2 and ot % 2 == 0) else "v"
                                        filler.append((350, mk(emit_oproj, 3, ot, qb, ev)))
                            return stage_mul
                        pending_recip.append(stage_recip)

                    def emit_pv_pair(g, half=half, segA=segA, segB=segB,
                                     e_tiles=e_tiles, cq=cq, stage0_norm=stage0_norm):
                        for kt in (2 * g, 2 * g + 1):
                            ensure(lambda: kt in v1_cur.get(half, {}))
                            if "A" not in cq:
                                cq["A"] = ps_c.tile([65, 512], F32, tag="ctxA", name="ctxA")
                                cq["B"] = ps_c.tile([65, 512], F32, tag="ctxB", name="ctxB")
                            v1v = v1_cur[half][kt][:].rearrange("p (s c) -> p s c", c=65)
                            e = e_tiles[kt]
                            nc.tensor.matmul(cq["A"][:], v1v[:, segA, :], e[:, 0:512],
                                             start=(kt == 0), stop=(kt == NKT - 1),
                                             skip_group_check=True)
                            nc.tensor.matmul(cq["B"][:], v1v[:, segB, :], e[:, 512:1024],
                                             start=(kt == 0), stop=(kt == NKT - 1),
                                             skip_group_check=True)
                        if g == 7:
                            stage0_norm()

                    # adaptive filler budget: aim to clear the queue within
                    # THIS pass (later passes bring their own work)
                    slots_left = (NQB - qb) * 8
                    pend = sum(e[0] for e in filler)
                    budget = max(600, min(1800, pend // max(slots_left, 1)))

                    for g in range(8):
                        # correctness barrier: Tile orders by emission, so
                        # the q/k projection groups feeding these scores
                        # must be fully emitted first
                        ensure(lambda: (p, "q", qb) in qk_done
                               and (p, "k", (2 * g) // 4) in qk_done
                               and (p, "k", (2 * g + 1) // 4) in qk_done)
                        emit_scores(2 * g)
                        emit_scores(2 * g + 1)
                        pending_pv.append(mk(emit_pv_pair, g))
                        if len(pending_pv) > 3:
                            pending_pv.popleft()()
                        drain(budget)

            # tail: flush PV pipeline, deferred normalize stages, then
            # remaining background work
            while pending_pv:
                pending_pv.popleft()()
                drain(700)
            while pending_recip:
                pending_mul.append(pending_recip.popleft()())
            while pending_mul:
                pending_mul.popleft()()
            while filler:
                filler.popleft()[1]()
    return nc


_nc_cache = None


def _get_nc():
    global _nc_cache
    if _nc_cache is None:
        _install_compile_patch()
        _nc_cache = _build()
    return _nc_cache


def _execute(inputs, trace=False, tmpdir=None):
    from concourse.bass_utils import run_bass_kernel_spmd

    bf = mybir.dt.np(BF16)
    x = np.asarray(inputs["x"], dtype=np.float32)
    Wq = np.asarray(inputs["Wq"], dtype=np.float32)
    Wk = np.asarray(inputs["Wk"], dtype=np.float32)
    Wv = np.asarray(inputs["Wv"], dtype=np.float32)
    Wo = np.asarray(inputs["Wo"], dtype=np.float32)
    bq = np.asarray(inputs["bq"], dtype=np.float32)
    bk = np.asarray(inputs["bk"], dtype=np.float32)
    bv = np.asarray(inputs["bv"], dtype=np.float32)
    bo = np.asarray(inputs["bo"], dtype=np.float32)

    def tile_qk(w):
        # [1024, 512] -> [128 partitions, pass*1024 + kt*128 + col]
        # partition i, pass p, kt k, col j = w[k*128+i, p*128+j]
        t = w.reshape(8, 128, 4, 128)          # [kt, i, p, j]
        return np.ascontiguousarray(t.transpose(1, 2, 0, 3).reshape(128, 4096))

    def tile_v(w):
        # [1024, 512] -> [128, half*2048 + kt*256 + col]
        t = w.reshape(8, 128, 2, 256)          # [kt, i, half, j]
        return np.ascontiguousarray(t.transpose(1, 2, 0, 3).reshape(128, 4096))

    nc = _get_nc()
    in_maps = []
    for c in range(N_CORES):
        b, g = c // 2, c % 2
        sl = slice(g * FT, (g + 1) * FT)
        bv_g = bv[sl].reshape(8, 64)
        bvones = np.concatenate(
            [bv_g, np.ones((8, 1), np.float32)], axis=1).reshape(-1)
        in_maps.append({
            "xT": np.ascontiguousarray(x[b].T).astype(bf),
            "wq": tile_qk(Wq[:, sl]).astype(bf),
            "wk": tile_qk(Wk[:, sl]).astype(bf),
            "wv": tile_v(Wv[:, sl]).astype(bf),
            "wo": np.ascontiguousarray(Wo[sl, :]).astype(bf),
            "bq": np.ascontiguousarray(bq[sl]),
            "bk": np.ascontiguousarray(bk[sl]),
            "bvones": bvones.astype(np.float32),
        })

    kwargs = {}
    if trace:
        kwargs = dict(trace=True, tmpdir=tmpdir)
    res = run_bass_kernel_spmd(nc, in_maps, core_ids=list(range(N_CORES)), **kwargs)

    out = np.empty((B, S, D), dtype=np.float32)
    for b in range(B):
        a0 = res.results[2 * b]["yTp"].reshape(NPASS, D, S).sum(axis=0)
        a1 = res.results[2 * b + 1]["yTp"].reshape(NPASS, D, S).sum(axis=0)
        out[b] = (a0 + a1).T + bo
    return out, res


def kernel(**inputs) -> np.ndarray:
    out, _ = _execute(inputs, trace=False)
    return out
